# revision 1
# baseline (speedup 1.0000x reference)
"""AttentionCommModule TRN2 kernel: 8-core data-parallel single-query MHA.

Sharding: batch B=32768 split across 8 NeuronCores (4096 rows each); all
weights replicated. Inputs are host-packed to bf16 in slab-chunk-major
layout so each core can xbar-DMA-transpose activation tiles straight from
DRAM into [k, b] SBUF layout (no on-chip transposes).

Per 128-row tile on device (batch-major layout, b on partitions):
  TensorE : Q/K/V projections, lhsT = transposed activation chunk
            (stationary), rhs = packed weights, f32 PSUM accumulation.
  ScalarE : PSUM -> SBUF copies (cast to bf16), exp().
  VectorE : QK dot-products + halving-tree d-reduction, softmax
            (batched per 4 tiles), attn*V + n-reduction.
  out-proj: weighted tile xbar-transposed (SBUF->SBUF DMA), one matmul
            against Wo^T, result staged in SBUF, DMA'd out per macro-tile.
"""

import numpy as np
import ml_dtypes
from contextlib import ExitStack

import concourse.bass as bass
import concourse.tile as tile
from concourse import bacc, mybir
from concourse.bass_utils import run_bass_kernel_spmd

N_CORES = 8
B_FULL = 32768
INPUT_DIM = 256
COMM = 128
NH = 4
HD = 32
N_MSGS = 7
NS = 8          # slabs = num_agents (self + 7 messages)
TILE = 128      # rows per attention tile
MACRO = 512     # rows per DMA macro-tile

BF = mybir.dt.bfloat16
F32 = mybir.dt.float32
INV_SQRT_HD = 1.0 / float(np.sqrt(HD))

_compiled = {}


def _build(bs: int, has_bias: bool):
    """Build + compile the per-core Bass program for a bs-row shard."""
    assert bs % MACRO == 0
    nc = bacc.Bacc(
        "TRN2",
        target_bir_lowering=False,
        debug=False,
        enable_asserts=False,
        num_devices=N_CORES,
    )
    xpack = nc.dram_tensor("xpack", [2 * NS, bs, 128], BF, kind="ExternalInput").ap()
    wpack = nc.dram_tensor("wpack", [128, 2, 384], BF, kind="ExternalInput").ap()
    wod = nc.dram_tensor("wo", [128, 128], BF, kind="ExternalInput").ap()
    if has_bias:
        bkvd = nc.dram_tensor("bkv", [1, 2176], F32, kind="ExternalInput").ap()
        bod = nc.dram_tensor("bo", [1, 128], F32, kind="ExternalInput").ap()
    outd = nc.dram_tensor("out", [bs, 128], F32, kind="ExternalOutput").ap()

    with tile.TileContext(nc) as tc, ExitStack() as ctx:
        consts = ctx.enter_context(tc.tile_pool(name="consts", bufs=1))
        xtp = ctx.enter_context(tc.tile_pool(name="xtp", bufs=2))
        kvp = ctx.enter_context(tc.tile_pool(name="kvp", bufs=8))
        work = ctx.enter_context(tc.tile_pool(name="work", bufs=3))
        sm = ctx.enter_context(tc.tile_pool(name="sm", bufs=2))
        osb = ctx.enter_context(tc.tile_pool(name="osb", bufs=2))
        psum = ctx.enter_context(tc.tile_pool(name="psum", bufs=8, space="PSUM"))

        w_sb = consts.tile([128, 768], BF)
        nc.scalar.dma_start(w_sb[:].rearrange("p (c n) -> p c n", c=2), wpack[:, :, :])
        wo_sb = consts.tile([128, 128], BF)
        nc.scalar.dma_start(wo_sb[:], wod[:, :])
        if has_bias:
            bkv_sb = consts.tile([1, 2176], F32)
            nc.scalar.dma_start(bkv_sb[:], bkvd[:, :])
            bo_sb = consts.tile([1, 128], F32)
            nc.scalar.dma_start(bo_sb[:], bod[:, :])

        n_macro = bs // MACRO
        for m in range(n_macro):
            # ---- input load: 16 xbar-transposing DMAs -> xt [k, b] ----
            xt = xtp.tile([128, 16 * MACRO], BF, name=f"xt{m}", tag="xt")
            for c in range(16):
                nc.sync.dma_start(
                    xt[:, c * MACRO:(c + 1) * MACRO],
                    xpack[c, m * MACRO:(m + 1) * MACRO, :],
                    transpose=True,
                )

            scores4 = sm.tile([128, 128], F32, tag="scores4")
            kv2s = []
            # ---- pass 1: QKV projections + K-scores ----
            for j in range(4):
                # PSUM layout: [Q | K0 V0 | K1 V1 ... K7 V7]
                psA = psum.tile([128, 384], F32, tag="ps", name=f"psA{m}_{j}")
                for ch in (0, 1):
                    nc.tensor.matmul(
                        psA[:, :],
                        lhsT=xt[:, ch * MACRO + j * TILE: ch * MACRO + j * TILE + 128],
                        rhs=w_sb[:, ch * 384:(ch + 1) * 384],
                        start=(ch == 0),
                        stop=(ch == 1),
                    )
                kvtiles = [psA]
                for pi, pair in enumerate(((1, 2), (3, 4), (5, 6), (7,))):
                    width = 256 * len(pair)
                    ps = psum.tile([128, width], F32, tag="ps", name=f"ps{pi}_{m}_{j}")
                    for si, s in enumerate(pair):
                        for ch in (0, 1):
                            nc.tensor.matmul(
                                ps[:, si * 256:(si + 1) * 256],
                                lhsT=xt[:, (2 * s + ch) * MACRO + j * TILE:
                                        (2 * s + ch) * MACRO + j * TILE + 128],
                                rhs=w_sb[:, ch * 384 + 128:(ch + 1) * 384],
                                start=(ch == 0),
                                stop=(ch == 1),
                            )
                    kvtiles.append(ps)

                kv2 = kvp.tile([128, 2176], BF, tag="kv2", name=f"kv2_{m}_{j}")
                kv2s.append(kv2)
                off = 0
                for ps in kvtiles:
                    w = ps.shape[1]
                    nc.scalar.copy(kv2[:, off:off + w], ps[:, :])
                    off += w
                if has_bias:
                    nc.vector.tensor_add(
                        kv2[:, :], kv2[:, :], bkv_sb[:, :].partition_broadcast(128)
                    )

                # K-products: q (bcast over slabs) * K_s
                qb = (
                    kv2[:, 0:128]
                    .rearrange("p (h d) -> p h d", h=NH)
                    .unsqueeze(1)
                    .broadcast_to([128, NS, NH, HD])
                )
                kk = kv2[:, 128:2176].rearrange(
                    "p (s kv h d) -> p s kv h d", s=NS, kv=2, h=NH
                )[:, :, 0, :, :]
                p1 = work.tile([128, 1024], BF, tag="p1", name=f"p1_{m}_{j}")
                nc.vector.tensor_mul(
                    p1[:].rearrange("p (s h d) -> p s h d", s=NS, h=NH), qb, kk
                )
                # d-reduction tree: 32 -> 1 per (s, h)
                cur, cd = p1, HD
                for r in range(4):
                    nxt = work.tile(
                        [128, NS * NH * cd // 2], BF, tag=f"t{r}", name=f"t{r}_{m}_{j}"
                    )
                    v = cur[:].rearrange(
                        "p (s h e d) -> p s h e d", s=NS, h=NH, e=2
                    )
                    nc.vector.tensor_add(
                        nxt[:].rearrange("p (s h d) -> p s h d", s=NS, h=NH),
                        v[:, :, :, 0, :],
                        v[:, :, :, 1, :],
                    )
                    cur, cd = nxt, cd // 2
                v = cur[:].rearrange("p (s h e) -> p s h e", s=NS, h=NH)
                nc.vector.tensor_add(
                    scores4[:, j * 32:(j + 1) * 32].rearrange(
                        "p (s h) -> p s h", s=NS
                    ),
                    v[:, :, :, 0],
                    v[:, :, :, 1],
                )

            # ---- softmax over slabs, batched for the 4 tiles ----
            # scores4 layout: (t, s, h) per partition
            sc_tsh = scores4[:].rearrange("p (t s h) -> p t s h", t=4, s=NS)
            sc_ths = sc_tsh.transpose([0, 1, 3, 2])
            mx = sm.tile([128, 16], F32, tag="mx")
            nc.vector.reduce_max(mx[:].rearrange("p (t h) -> p t h", t=4),
                                 sc_ths, axis=mybir.AxisListType.X)
            u4 = sm.tile([128, 128], F32, tag="u4")
            mxb = (
                mx[:]
                .rearrange("p (t h) -> p t h", t=4)
                .unsqueeze(2)
                .broadcast_to([128, 4, NS, NH])
            )
            nc.vector.tensor_sub(
                u4[:].rearrange("p (t s h) -> p t s h", t=4, s=NS), sc_tsh, mxb
            )
            e4 = sm.tile([128, 128], BF, tag="e4")
            nc.scalar.activation(
                e4[:], u4[:], mybir.ActivationFunctionType.Exp, scale=INV_SQRT_HD
            )
            s4 = sm.tile([128, 16], F32, tag="s4")
            e_tsh = e4[:].rearrange("p (t s h) -> p t s h", t=4, s=NS)
            nc.vector.reduce_sum(s4[:].rearrange("p (t h) -> p t h", t=4),
                                 e_tsh.transpose([0, 1, 3, 2]),
                                 axis=mybir.AxisListType.X)
            r4 = sm.tile([128, 16], F32, tag="r4")
            nc.vector.reciprocal(r4[:], s4[:])
            a4 = sm.tile([128, 128], BF, tag="a4")
            r4b = (
                r4[:]
                .rearrange("p (t h) -> p t h", t=4)
                .unsqueeze(2)
                .broadcast_to([128, 4, NS, NH])
            )
            nc.vector.tensor_mul(
                a4[:].rearrange("p (t s h) -> p t s h", t=4, s=NS), e_tsh, r4b
            )

            # ---- pass 2: attn * V, n-reduction, out-projection ----
            out_sb = osb.tile([128, 4 * TILE], F32, tag="out_sb", name=f"osb{m}")
            for j in range(4):
                kv2 = kv2s[j]
                ab = (
                    a4[:, j * 32:(j + 1) * 32]
                    .rearrange("p (s h) -> p s h", s=NS)
                    .unsqueeze(3)
                    .broadcast_to([128, NS, NH, HD])
                )
                vv = kv2[:, 128:2176].rearrange(
                    "p (s kv h d) -> p s kv h d", s=NS, kv=2, h=NH
                )[:, :, 1, :, :]
                p2 = work.tile([128, 1024], BF, tag="p2", name=f"p2_{m}_{j}")
                nc.vector.tensor_mul(
                    p2[:].rearrange("p (s h d) -> p s h d", s=NS, h=NH), ab, vv
                )
                cur, cn = p2, NS
                for r in range(3):
                    nxt = work.tile(
                        [128, cn * 64], BF, tag=f"v{r}", name=f"v{r}_{m}_{j}"
                    )
                    nc.vector.tensor_add(
                        nxt[:], cur[:, 0:cn * 64], cur[:, cn * 64:cn * 128]
                    )
                    cur, cn = nxt, cn // 2
                wtd = cur  # [128, 128] bf16, batch-major
                wtdT = work.tile([128, 128], BF, tag="wtdT", name=f"wtdT_{m}_{j}")
                nc.sync.dma_start(wtdT[:], wtd[:], transpose=True)
                po = psum.tile([128, 128], F32, tag="ps", name=f"po_{m}_{j}")
                nc.tensor.matmul(po[:], lhsT=wtdT[:], rhs=wo_sb[:],
                                 start=True, stop=True)
                if has_bias:
                    nc.vector.tensor_add(
                        out_sb[:, j * 128:(j + 1) * 128], po[:],
                        bo_sb[:, :].partition_broadcast(128),
                    )
                else:
                    nc.scalar.copy(out_sb[:, j * 128:(j + 1) * 128], po[:])

            nc.scalar.dma_start(
                outd[m * MACRO:(m + 1) * MACRO, :].rearrange(
                    "(t p) j -> p t j", t=4
                ),
                out_sb[:].rearrange("p (t j) -> p t j", t=4),
            )

    nc.compile()
    return nc


def _get_compiled(bs: int, has_bias: bool):
    key = (bs, has_bias)
    if key not in _compiled:
        _compiled[key] = _build(bs, has_bias)
    return _compiled[key]


def _pack_inputs(agent_obs, messages, Wq, bq, Wk, bk, Wv, bv, Wo, bo):
    """Host-side packing (per full batch): returns dict of device arrays."""
    bf16 = ml_dtypes.bfloat16
    b = agent_obs.shape[0]
    allm = np.concatenate([agent_obs[:, None, :], messages], axis=1)  # [b, 8, 256]
    # slab-chunk-major: xpack[2s+ch, b, :] = slab s cols [128ch:128ch+128]
    xpack = np.ascontiguousarray(
        allm.reshape(b, NS, 2, 128).transpose(1, 2, 0, 3).reshape(16, b, 128)
    ).astype(bf16)

    wcat = np.concatenate([Wq.T, Wk.T, Wv.T], axis=1)  # [256, 384]
    wpack = np.ascontiguousarray(
        wcat.reshape(2, 128, 384).transpose(1, 0, 2)
    ).astype(bf16)  # [128, 2, 384]
    wo = np.ascontiguousarray(Wo.T).astype(bf16)  # [128, 128]

    has_bias = bool(
        np.any(bq != 0) or np.any(bk != 0) or np.any(bv != 0) or np.any(bo != 0)
    )
    extra = {}
    if has_bias:
        # PSUM layout [Q | K0 V0 | ... | K7 V7]
        bkv = np.zeros((1, 2176), np.float32)
        bkv[0, 0:128] = bq
        for s in range(NS):
            bkv[0, 128 + s * 256:128 + s * 256 + 128] = bk
            bkv[0, 256 + s * 256:256 + s * 256 + 128] = bv
        extra["bkv"] = bkv
        extra["bo"] = bo.reshape(1, 128).astype(np.float32)
    return xpack, wpack, wo, extra, has_bias


def kernel(agent_obs, messages, Wq, bq, Wk, bk, Wv, bv, Wo, bo):
    b = agent_obs.shape[0]
    assert b % N_CORES == 0
    bs = b // N_CORES

    xpack, wpack, wo, extra, has_bias = _pack_inputs(
        np.asarray(agent_obs, np.float32), np.asarray(messages, np.float32),
        np.asarray(Wq, np.float32), np.asarray(bq, np.float32),
        np.asarray(Wk, np.float32), np.asarray(bk, np.float32),
        np.asarray(Wv, np.float32), np.asarray(bv, np.float32),
        np.asarray(Wo, np.float32), np.asarray(bo, np.float32),
    )
    nc = _get_compiled(bs, has_bias)

    in_maps = []
    for c in range(N_CORES):
        m = {
            "xpack": np.ascontiguousarray(xpack[:, c * bs:(c + 1) * bs, :]),
            "wpack": wpack,
            "wo": wo,
        }
        m.update(extra)
        in_maps.append(m)

    res = run_bass_kernel_spmd(nc, in_maps, core_ids=list(range(N_CORES)))
    out = np.concatenate([r["out"] for r in res.results], axis=0)
    return out.astype(np.float32)


# revision 10
# speedup vs baseline: 1.1405x; 1.1405x over previous
"""AttentionCommModule TRN2 kernel: 8-core data-parallel single-query MHA.

Sharding: batch B=32768 split across 8 NeuronCores (4096 rows each); all
weights replicated. Inputs are host-packed to bf16 in slab-chunk-major
layout so each core can xbar-DMA-transpose activation tiles straight from
DRAM into [k, b] SBUF layout (no on-chip transposes).

Per 128-row tile on device (batch-major layout, b on partitions):
  TensorE : Q/K/V projections, lhsT = transposed activation chunk
            (stationary), rhs = packed weights, f32 PSUM accumulation.
  ScalarE : PSUM -> SBUF copies (cast to bf16), exp().
  VectorE : QK dot-products + halving-tree d-reduction, softmax
            (batched per 4 tiles), attn*V + n-reduction.
  out-proj: weighted tile xbar-transposed (SBUF->SBUF DMA), one matmul
            against Wo^T, result staged in SBUF, DMA'd out per macro-tile.
"""

import numpy as np
import ml_dtypes
from contextlib import ExitStack

import concourse.bass as bass
import concourse.tile as tile
from concourse import bacc, mybir
from concourse.bass_utils import run_bass_kernel_spmd

N_CORES = 8
B_FULL = 32768
INPUT_DIM = 256
COMM = 128
NH = 4
HD = 32
N_MSGS = 7
NS = 8          # slabs = num_agents (self + 7 messages)
TILE = 128      # rows per attention tile
MACRO = 512     # rows per DMA macro-tile

BF = mybir.dt.bfloat16
F32 = mybir.dt.float32
INV_SQRT_HD = 1.0 / float(np.sqrt(HD))
PSUM_OUT_DMA = False  # PSUM is not DMA-addressable on this stack

_compiled = {}


def _build(bs: int, has_bias: bool):
    """Build + compile the per-core Bass program for a bs-row shard."""
    assert bs % MACRO == 0
    nc = bacc.Bacc(
        "TRN2",
        target_bir_lowering=False,
        debug=False,
        enable_asserts=False,
        num_devices=N_CORES,
    )
    # ktpack[c, k, b]: slab-chunk c, feature k on what becomes the SBUF
    # partition dim, batch contiguous — host pre-transposed.
    xpack = nc.dram_tensor("xpack", [2 * NS, 128, bs], BF, kind="ExternalInput").ap()
    wpack = nc.dram_tensor("wpack", [128, 2, 384], BF, kind="ExternalInput").ap()
    wod = nc.dram_tensor("wo", [128, 128], BF, kind="ExternalInput").ap()
    if has_bias:
        bkvd = nc.dram_tensor("bkv", [1, 2176], F32, kind="ExternalInput").ap()
        bod = nc.dram_tensor("bo", [1, 128], F32, kind="ExternalInput").ap()
    outd = nc.dram_tensor("out", [bs, 128], F32, kind="ExternalOutput").ap()

    with tile.TileContext(nc) as tc, ExitStack() as ctx:
        consts = ctx.enter_context(tc.tile_pool(name="consts", bufs=1))
        xtp = ctx.enter_context(tc.tile_pool(name="xtp", bufs=2))
        kvp = ctx.enter_context(tc.tile_pool(name="kvp", bufs=8))
        work = ctx.enter_context(tc.tile_pool(name="work", bufs=3))
        sm = ctx.enter_context(tc.tile_pool(name="sm", bufs=2))
        osb = ctx.enter_context(tc.tile_pool(name="osb", bufs=2))
        psum = ctx.enter_context(tc.tile_pool(name="psum", bufs=8, space="PSUM"))

        w_sb = consts.tile([128, 768], BF)
        nc.scalar.dma_start(w_sb[:].rearrange("p (c n) -> p c n", c=2), wpack[:, :, :])
        wo_sb = consts.tile([128, 128], BF)
        nc.scalar.dma_start(wo_sb[:], wod[:, :])
        if has_bias:
            bkv_sb = consts.tile([1, 2176], F32)
            nc.scalar.dma_start(bkv_sb[:], bkvd[:, :])
            bo_sb = consts.tile([1, 128], F32)
            nc.scalar.dma_start(bo_sb[:], bod[:, :])

        n_macro = bs // MACRO
        for m in range(n_macro):
            # ---- input load: 16 plain DMAs, already [k, b] in DRAM ----
            xt = xtp.tile([128, 16 * MACRO], BF, name=f"xt{m}", tag="xt")
            for c in range(16):
                nc.scalar.dma_start(
                    xt[:, c * MACRO:(c + 1) * MACRO],
                    xpack[c, :, m * MACRO:(m + 1) * MACRO],
                )

            scores4 = sm.tile([128, 128], F32, tag="scores4")
            kv2s = []
            # ---- pass 1: QKV projections + K-scores ----
            for j in range(4):
                # PSUM layout: [Q | K0 V0 | K1 V1 ... K7 V7]
                psA = psum.tile([128, 384], F32, tag="ps", name=f"psA{m}_{j}")
                for ch in (0, 1):
                    nc.tensor.matmul(
                        psA[:, :],
                        lhsT=xt[:, ch * MACRO + j * TILE: ch * MACRO + j * TILE + 128],
                        rhs=w_sb[:, ch * 384:(ch + 1) * 384],
                        start=(ch == 0),
                        stop=(ch == 1),
                    )
                kvtiles = [psA]
                for pi, pair in enumerate(((1, 2), (3, 4), (5, 6), (7,))):
                    width = 256 * len(pair)
                    ps = psum.tile([128, width], F32, tag="ps", name=f"ps{pi}_{m}_{j}")
                    for si, s in enumerate(pair):
                        for ch in (0, 1):
                            nc.tensor.matmul(
                                ps[:, si * 256:(si + 1) * 256],
                                lhsT=xt[:, (2 * s + ch) * MACRO + j * TILE:
                                        (2 * s + ch) * MACRO + j * TILE + 128],
                                rhs=w_sb[:, ch * 384 + 128:(ch + 1) * 384],
                                start=(ch == 0),
                                stop=(ch == 1),
                            )
                    kvtiles.append(ps)

                kv2 = kvp.tile([128, 2176], BF, tag="kv2", name=f"kv2_{m}_{j}")
                kv2s.append(kv2)
                off = 0
                for ps in kvtiles:
                    w = ps.shape[1]
                    nc.scalar.copy(kv2[:, off:off + w], ps[:, :])
                    off += w
                if has_bias:
                    nc.vector.tensor_add(
                        kv2[:, :], kv2[:, :], bkv_sb[:, :].partition_broadcast(128)
                    )

                # K-products: q (bcast over slabs) * K_s
                qb = (
                    kv2[:, 0:128]
                    .rearrange("p (h d) -> p h d", h=NH)
                    .unsqueeze(1)
                    .broadcast_to([128, NS, NH, HD])
                )
                kk = kv2[:, 128:2176].rearrange(
                    "p (s kv h d) -> p s kv h d", s=NS, kv=2, h=NH
                )[:, :, 0, :, :]
                p1 = work.tile([128, 1024], BF, tag="p1", name=f"p1_{m}_{j}")
                nc.vector.tensor_mul(
                    p1[:].rearrange("p (s h d) -> p s h d", s=NS, h=NH), qb, kk
                )
                # d-reduction tree: 32 -> 1 per (s, h)
                cur, cd = p1, HD
                for r in range(4):
                    nxt = work.tile(
                        [128, NS * NH * cd // 2], BF, tag=f"t{r}", name=f"t{r}_{m}_{j}"
                    )
                    v = cur[:].rearrange(
                        "p (s h e d) -> p s h e d", s=NS, h=NH, e=2
                    )
                    nc.vector.tensor_add(
                        nxt[:].rearrange("p (s h d) -> p s h d", s=NS, h=NH),
                        v[:, :, :, 0, :],
                        v[:, :, :, 1, :],
                    )
                    cur, cd = nxt, cd // 2
                v = cur[:].rearrange("p (s h e) -> p s h e", s=NS, h=NH)
                nc.vector.tensor_add(
                    scores4[:, j * 32:(j + 1) * 32].rearrange(
                        "p (s h) -> p s h", s=NS
                    ),
                    v[:, :, :, 0],
                    v[:, :, :, 1],
                )

            # ---- softmax over slabs, batched for the 4 tiles ----
            # scores4 layout: (t, s, h) per partition
            sc_tsh = scores4[:].rearrange("p (t s h) -> p t s h", t=4, s=NS)
            sc_ths = sc_tsh.transpose([0, 1, 3, 2])
            mx = sm.tile([128, 16], F32, tag="mx")
            nc.vector.reduce_max(mx[:].rearrange("p (t h) -> p t h", t=4),
                                 sc_ths, axis=mybir.AxisListType.X)
            u4 = sm.tile([128, 128], F32, tag="u4")
            mxb = (
                mx[:]
                .rearrange("p (t h) -> p t h", t=4)
                .unsqueeze(2)
                .broadcast_to([128, 4, NS, NH])
            )
            nc.vector.tensor_sub(
                u4[:].rearrange("p (t s h) -> p t s h", t=4, s=NS), sc_tsh, mxb
            )
            e4 = sm.tile([128, 128], BF, tag="e4")
            nc.scalar.activation(
                e4[:], u4[:], mybir.ActivationFunctionType.Exp, scale=INV_SQRT_HD
            )
            s4 = sm.tile([128, 16], F32, tag="s4")
            e_tsh = e4[:].rearrange("p (t s h) -> p t s h", t=4, s=NS)
            nc.vector.reduce_sum(s4[:].rearrange("p (t h) -> p t h", t=4),
                                 e_tsh.transpose([0, 1, 3, 2]),
                                 axis=mybir.AxisListType.X)
            r4 = sm.tile([128, 16], F32, tag="r4")
            nc.vector.reciprocal(r4[:], s4[:])
            a4 = sm.tile([128, 128], BF, tag="a4")
            r4b = (
                r4[:]
                .rearrange("p (t h) -> p t h", t=4)
                .unsqueeze(2)
                .broadcast_to([128, 4, NS, NH])
            )
            nc.vector.tensor_mul(
                a4[:].rearrange("p (t s h) -> p t s h", t=4, s=NS), e_tsh, r4b
            )

            # ---- pass 2: attn * V, n-reduction, out-projection ----
            # V columns are d-major (c' = d*4 + h, host-permuted) so the
            # attn broadcast lands on a stride-1 innermost dim (DVE 2x).
            if not PSUM_OUT_DMA:
                out_sb = osb.tile([128, 4 * TILE], F32, tag="out_sb",
                                  name=f"osb{m}")
            for j in range(4):
                kv2 = kv2s[j]
                ab = (
                    a4[:, j * 32:(j + 1) * 32]
                    .rearrange("p (s h) -> p s h", s=NS)
                    .unsqueeze(2)
                    .broadcast_to([128, NS, HD, NH])
                )
                vv = kv2[:, 128:2176].rearrange(
                    "p (s kv d h) -> p s kv d h", s=NS, kv=2, d=HD
                )[:, :, 1, :, :]
                p2 = work.tile([128, 1024], BF, tag="p2", name=f"p2_{m}_{j}")
                nc.vector.tensor_mul(
                    p2[:].rearrange("p (s d h) -> p s d h", s=NS, d=HD), ab, vv
                )
                cur, cn = p2, NS
                for r in range(3):
                    nxt = work.tile(
                        [128, cn * 64], BF, tag=f"v{r}", name=f"v{r}_{m}_{j}"
                    )
                    nc.vector.tensor_add(
                        nxt[:], cur[:, 0:cn * 64], cur[:, cn * 64:cn * 128]
                    )
                    cur, cn = nxt, cn // 2
                wtd = cur  # [128, 128] bf16, batch-major, (d, h) cols
                wtdT = work.tile([128, 128], BF, tag="wtdT", name=f"wtdT_{m}_{j}")
                nc.sync.dma_start(wtdT[:], wtd[:], transpose=True)
                po = psum.tile([128, 128], F32, tag="ps", name=f"po_{m}_{j}")
                nc.tensor.matmul(po[:], lhsT=wtdT[:], rhs=wo_sb[:],
                                 start=True, stop=True)
                r0 = (m * 4 + j) * TILE
                if has_bias:
                    ob = osb.tile([128, 128], F32, tag="ob", name=f"ob_{m}_{j}")
                    nc.vector.tensor_add(
                        ob[:], po[:], bo_sb[:, :].partition_broadcast(128)
                    )
                    nc.scalar.dma_start(outd[r0:r0 + TILE, :], ob[:])
                elif PSUM_OUT_DMA:
                    nc.scalar.dma_start(outd[r0:r0 + TILE, :], po[:])
                else:
                    nc.scalar.copy(out_sb[:, j * 128:(j + 1) * 128], po[:])

            if not PSUM_OUT_DMA and not has_bias:
                nc.scalar.dma_start(
                    outd[m * MACRO:(m + 1) * MACRO, :].rearrange(
                        "(t p) j -> p t j", t=4
                    ),
                    out_sb[:].rearrange("p (t j) -> p t j", t=4),
                )

    nc.compile()
    return nc


def _get_compiled(bs: int, has_bias: bool):
    key = (bs, has_bias)
    if key not in _compiled:
        _compiled[key] = _build(bs, has_bias)
    return _compiled[key]


def _pack_inputs(agent_obs, messages, Wq, bq, Wk, bk, Wv, bv, Wo, bo):
    """Host-side packing (per full batch): returns dict of device arrays."""
    bf16 = ml_dtypes.bfloat16
    b = agent_obs.shape[0]
    allm = np.concatenate([agent_obs[:, None, :], messages], axis=1)  # [b, 8, 256]
    # slab-chunk-major, feature-transposed: xpack[2s+ch, k, b]
    xpack = np.ascontiguousarray(
        allm.reshape(b, NS, 2, 128).transpose(1, 2, 3, 0).reshape(16, 128, b)
    ).astype(bf16)

    # V (and Wo rows) in d-major column order c' = d*NH + h so the DVE
    # attn broadcast is stride-1 innermost.
    perm = (np.arange(128).reshape(NH, HD).T).reshape(-1)  # c' -> h*HD+d
    WvTp = Wv.T[:, perm]
    wcat = np.concatenate([Wq.T, Wk.T, WvTp], axis=1)  # [256, 384]
    wpack = np.ascontiguousarray(
        wcat.reshape(2, 128, 384).transpose(1, 0, 2)
    ).astype(bf16)  # [128, 2, 384]
    wo = np.ascontiguousarray(Wo.T[perm, :]).astype(bf16)  # [128, 128]

    has_bias = bool(
        np.any(bq != 0) or np.any(bk != 0) or np.any(bv != 0) or np.any(bo != 0)
    )
    extra = {}
    if has_bias:
        # PSUM layout [Q | K0 V0 | ... | K7 V7]
        bkv = np.zeros((1, 2176), np.float32)
        bkv[0, 0:128] = bq
        for s in range(NS):
            bkv[0, 128 + s * 256:128 + s * 256 + 128] = bk
            bkv[0, 256 + s * 256:256 + s * 256 + 128] = bv[perm]
        extra["bkv"] = bkv
        extra["bo"] = bo.reshape(1, 128).astype(np.float32)
    return xpack, wpack, wo, extra, has_bias


def kernel(agent_obs, messages, Wq, bq, Wk, bk, Wv, bv, Wo, bo):
    b = agent_obs.shape[0]
    assert b % N_CORES == 0
    bs = b // N_CORES

    xpack, wpack, wo, extra, has_bias = _pack_inputs(
        np.asarray(agent_obs, np.float32), np.asarray(messages, np.float32),
        np.asarray(Wq, np.float32), np.asarray(bq, np.float32),
        np.asarray(Wk, np.float32), np.asarray(bk, np.float32),
        np.asarray(Wv, np.float32), np.asarray(bv, np.float32),
        np.asarray(Wo, np.float32), np.asarray(bo, np.float32),
    )
    nc = _get_compiled(bs, has_bias)

    in_maps = []
    for c in range(N_CORES):
        m = {
            "xpack": np.ascontiguousarray(xpack[:, :, c * bs:(c + 1) * bs]),
            "wpack": wpack,
            "wo": wo,
        }
        m.update(extra)
        in_maps.append(m)

    res = run_bass_kernel_spmd(nc, in_maps, core_ids=list(range(N_CORES)))
    out = np.concatenate([r["out"] for r in res.results], axis=0)
    return out.astype(np.float32)


# revision 13
# speedup vs baseline: 1.1673x; 1.0235x over previous
"""AttentionCommModule TRN2 kernel: 8-core data-parallel single-query MHA.

Sharding: batch B=32768 split across 8 NeuronCores (4096 rows each); all
weights replicated. Inputs are host-packed to bf16 in slab-chunk-major
layout so each core can xbar-DMA-transpose activation tiles straight from
DRAM into [k, b] SBUF layout (no on-chip transposes).

Per 128-row tile on device (batch-major layout, b on partitions):
  TensorE : Q/K/V projections, lhsT = transposed activation chunk
            (stationary), rhs = packed weights, f32 PSUM accumulation.
  ScalarE : PSUM -> SBUF copies (cast to bf16), exp().
  VectorE : QK dot-products + halving-tree d-reduction, softmax
            (batched per 4 tiles), attn*V + n-reduction.
  out-proj: weighted tile xbar-transposed (SBUF->SBUF DMA), one matmul
            against Wo^T, result staged in SBUF, DMA'd out per macro-tile.
"""

import numpy as np
import ml_dtypes
from contextlib import ExitStack

import concourse.bass as bass
import concourse.tile as tile
from concourse import bacc, mybir
from concourse.bass_utils import run_bass_kernel_spmd

N_CORES = 8
B_FULL = 32768
INPUT_DIM = 256
COMM = 128
NH = 4
HD = 32
N_MSGS = 7
NS = 8          # slabs = num_agents (self + 7 messages)
TILE = 128      # rows per attention tile
MACRO = 512     # rows per DMA macro-tile

BF = mybir.dt.bfloat16
F32 = mybir.dt.float32
INV_SQRT_HD = 1.0 / float(np.sqrt(HD))
PSUM_OUT_DMA = False  # PSUM is not DMA-addressable on this stack

_compiled = {}


def _build(bs: int, has_bias: bool):
    """Build + compile the per-core Bass program for a bs-row shard."""
    assert bs % MACRO == 0
    nc = bacc.Bacc(
        "TRN2",
        target_bir_lowering=False,
        debug=False,
        enable_asserts=False,
        num_devices=N_CORES,
    )
    # ktpack[c, k, b]: slab-chunk c, feature k on what becomes the SBUF
    # partition dim, batch contiguous — host pre-transposed.
    xpack = nc.dram_tensor("xpack", [2 * NS, 128, bs], BF, kind="ExternalInput").ap()
    wpack = nc.dram_tensor("wpack", [128, 2, 384], BF, kind="ExternalInput").ap()
    wod = nc.dram_tensor("wo", [128, 128], BF, kind="ExternalInput").ap()
    if has_bias:
        bkvd = nc.dram_tensor("bkv", [1, 2176], F32, kind="ExternalInput").ap()
        bod = nc.dram_tensor("bo", [1, 128], F32, kind="ExternalInput").ap()
    outd = nc.dram_tensor("out", [bs, 128], F32, kind="ExternalOutput").ap()

    with tile.TileContext(nc) as tc, ExitStack() as ctx:
        consts = ctx.enter_context(tc.tile_pool(name="consts", bufs=1))
        xtp = ctx.enter_context(tc.tile_pool(name="xtp", bufs=3))
        kvp = ctx.enter_context(tc.tile_pool(name="kvp", bufs=12))
        work = ctx.enter_context(tc.tile_pool(name="work", bufs=4))
        sm = ctx.enter_context(tc.tile_pool(name="sm", bufs=3))
        osb = ctx.enter_context(tc.tile_pool(name="osb", bufs=2))
        psum = ctx.enter_context(tc.tile_pool(name="psum", bufs=8, space="PSUM"))

        w_sb = consts.tile([128, 768], BF)
        nc.gpsimd.dma_start(w_sb[:].rearrange("p (c n) -> p c n", c=2), wpack[:, :, :])
        wo_sb = consts.tile([128, 128], BF)
        nc.gpsimd.dma_start(wo_sb[:], wod[:, :])
        if has_bias:
            bkv_sb = consts.tile([1, 2176], F32)
            nc.gpsimd.dma_start(bkv_sb[:], bkvd[:, :])
            bo_sb = consts.tile([1, 128], F32)
            nc.gpsimd.dma_start(bo_sb[:], bod[:, :])

        n_macro = bs // MACRO
        # software pipeline: emit pass-2 of macro m-1 after pass-1 of m,
        # so no engine's static program order stalls on the cross-engine
        # attention chain at macro boundaries.
        pending_pass2 = [None]

        def emit_pass1(m):
            # ---- input load: 16 plain DMAs, already [k, b] in DRAM ----
            xt = xtp.tile([128, 16 * MACRO], BF, name=f"xt{m}", tag="xt")
            for c in range(16):
                nc.scalar.dma_start(
                    xt[:, c * MACRO:(c + 1) * MACRO],
                    xpack[c, :, m * MACRO:(m + 1) * MACRO],
                )

            scores4 = sm.tile([128, 128], F32, tag="scores4")
            kv2s = []
            # ---- pass 1: QKV projections + K-scores ----
            for j in range(4):
                # PSUM layout: [Q | K0 V0 | K1 V1 ... K7 V7]
                psA = psum.tile([128, 384], F32, tag="ps", name=f"psA{m}_{j}")
                for ch in (0, 1):
                    nc.tensor.matmul(
                        psA[:, :],
                        lhsT=xt[:, ch * MACRO + j * TILE: ch * MACRO + j * TILE + 128],
                        rhs=w_sb[:, ch * 384:(ch + 1) * 384],
                        start=(ch == 0),
                        stop=(ch == 1),
                    )
                kvtiles = [psA]
                for pi, pair in enumerate(((1, 2), (3, 4), (5, 6), (7,))):
                    width = 256 * len(pair)
                    ps = psum.tile([128, width], F32, tag="ps", name=f"ps{pi}_{m}_{j}")
                    for si, s in enumerate(pair):
                        for ch in (0, 1):
                            nc.tensor.matmul(
                                ps[:, si * 256:(si + 1) * 256],
                                lhsT=xt[:, (2 * s + ch) * MACRO + j * TILE:
                                        (2 * s + ch) * MACRO + j * TILE + 128],
                                rhs=w_sb[:, ch * 384 + 128:(ch + 1) * 384],
                                start=(ch == 0),
                                stop=(ch == 1),
                            )
                    kvtiles.append(ps)

                kv2 = kvp.tile([128, 2176], BF, tag="kv2", name=f"kv2_{m}_{j}")
                kv2s.append(kv2)
                off = 0
                for ps in kvtiles:
                    w = ps.shape[1]
                    nc.scalar.copy(kv2[:, off:off + w], ps[:, :])
                    off += w
                if has_bias:
                    nc.vector.tensor_add(
                        kv2[:, :], kv2[:, :], bkv_sb[:, :].partition_broadcast(128)
                    )

                # K-products: q (bcast over slabs) * K_s
                qb = (
                    kv2[:, 0:128]
                    .rearrange("p (h d) -> p h d", h=NH)
                    .unsqueeze(1)
                    .broadcast_to([128, NS, NH, HD])
                )
                kk = kv2[:, 128:2176].rearrange(
                    "p (s kv h d) -> p s kv h d", s=NS, kv=2, h=NH
                )[:, :, 0, :, :]
                p1 = work.tile([128, 1024], BF, tag="p1", name=f"p1_{m}_{j}")
                nc.vector.tensor_mul(
                    p1[:].rearrange("p (s h d) -> p s h d", s=NS, h=NH), qb, kk
                )
                # d-reduction tree: 32 -> 1 per (s, h)
                cur, cd = p1, HD
                for r in range(4):
                    nxt = work.tile(
                        [128, NS * NH * cd // 2], BF, tag=f"t{r}", name=f"t{r}_{m}_{j}"
                    )
                    v = cur[:].rearrange(
                        "p (s h e d) -> p s h e d", s=NS, h=NH, e=2
                    )
                    nc.vector.tensor_add(
                        nxt[:].rearrange("p (s h d) -> p s h d", s=NS, h=NH),
                        v[:, :, :, 0, :],
                        v[:, :, :, 1, :],
                    )
                    cur, cd = nxt, cd // 2
                v = cur[:].rearrange("p (s h e) -> p s h e", s=NS, h=NH)
                nc.vector.tensor_add(
                    scores4[:, j * 32:(j + 1) * 32].rearrange(
                        "p (s h) -> p s h", s=NS
                    ),
                    v[:, :, :, 0],
                    v[:, :, :, 1],
                )

            # ---- softmax over slabs, batched for the 4 tiles ----
            # scores4 layout: (t, s, h) per partition
            sc_tsh = scores4[:].rearrange("p (t s h) -> p t s h", t=4, s=NS)
            sc_ths = sc_tsh.transpose([0, 1, 3, 2])
            mx = sm.tile([128, 16], F32, tag="mx")
            nc.vector.reduce_max(mx[:].rearrange("p (t h) -> p t h", t=4),
                                 sc_ths, axis=mybir.AxisListType.X)
            u4 = sm.tile([128, 128], F32, tag="u4")
            mxb = (
                mx[:]
                .rearrange("p (t h) -> p t h", t=4)
                .unsqueeze(2)
                .broadcast_to([128, 4, NS, NH])
            )
            nc.vector.tensor_sub(
                u4[:].rearrange("p (t s h) -> p t s h", t=4, s=NS), sc_tsh, mxb
            )
            e4 = sm.tile([128, 128], BF, tag="e4")
            nc.scalar.activation(
                e4[:], u4[:], mybir.ActivationFunctionType.Exp, scale=INV_SQRT_HD
            )
            s4 = sm.tile([128, 16], F32, tag="s4")
            e_tsh = e4[:].rearrange("p (t s h) -> p t s h", t=4, s=NS)
            nc.vector.reduce_sum(s4[:].rearrange("p (t h) -> p t h", t=4),
                                 e_tsh.transpose([0, 1, 3, 2]),
                                 axis=mybir.AxisListType.X)
            r4 = sm.tile([128, 16], F32, tag="r4")
            nc.vector.reciprocal(r4[:], s4[:])
            a4 = sm.tile([128, 128], BF, tag="a4")
            r4b = (
                r4[:]
                .rearrange("p (t h) -> p t h", t=4)
                .unsqueeze(2)
                .broadcast_to([128, 4, NS, NH])
            )
            nc.vector.tensor_mul(
                a4[:].rearrange("p (t s h) -> p t s h", t=4, s=NS), e_tsh, r4b
            )
            return kv2s, a4

        def emit_pass2(m, kv2s, a4):
            # ---- pass 2: attn * V, n-reduction, out-projection ----
            # V columns are d-major (c' = d*4 + h, host-permuted) so the
            # attn broadcast lands on a stride-1 innermost dim (DVE 2x).
            if not PSUM_OUT_DMA:
                out_sb = osb.tile([128, 4 * TILE], F32, tag="out_sb",
                                  name=f"osb{m}")
            for j in range(4):
                kv2 = kv2s[j]
                ab = (
                    a4[:, j * 32:(j + 1) * 32]
                    .rearrange("p (s h) -> p s h", s=NS)
                    .unsqueeze(2)
                    .broadcast_to([128, NS, HD, NH])
                )
                vv = kv2[:, 128:2176].rearrange(
                    "p (s kv d h) -> p s kv d h", s=NS, kv=2, d=HD
                )[:, :, 1, :, :]
                p2 = work.tile([128, 1024], BF, tag="p2", name=f"p2_{m}_{j}")
                nc.vector.tensor_mul(
                    p2[:].rearrange("p (s d h) -> p s d h", s=NS, d=HD), ab, vv
                )
                cur, cn = p2, NS
                for r in range(3):
                    nxt = work.tile(
                        [128, cn * 64], BF, tag=f"v{r}", name=f"v{r}_{m}_{j}"
                    )
                    nc.vector.tensor_add(
                        nxt[:], cur[:, 0:cn * 64], cur[:, cn * 64:cn * 128]
                    )
                    cur, cn = nxt, cn // 2
                wtd = cur  # [128, 128] bf16, batch-major, (d, h) cols
                wtdT = work.tile([128, 128], BF, tag="wtdT", name=f"wtdT_{m}_{j}")
                nc.sync.dma_start(wtdT[:], wtd[:], transpose=True)
                po = psum.tile([128, 128], F32, tag="ps", name=f"po_{m}_{j}")
                nc.tensor.matmul(po[:], lhsT=wtdT[:], rhs=wo_sb[:],
                                 start=True, stop=True)
                r0 = (m * 4 + j) * TILE
                if has_bias:
                    ob = osb.tile([128, 128], F32, tag="ob", name=f"ob_{m}_{j}")
                    nc.vector.tensor_add(
                        ob[:], po[:], bo_sb[:, :].partition_broadcast(128)
                    )
                    nc.gpsimd.dma_start(outd[r0:r0 + TILE, :], ob[:])
                elif PSUM_OUT_DMA:
                    nc.gpsimd.dma_start(outd[r0:r0 + TILE, :], po[:])
                else:
                    nc.scalar.copy(out_sb[:, j * 128:(j + 1) * 128], po[:])

            if not PSUM_OUT_DMA and not has_bias:
                nc.gpsimd.dma_start(
                    outd[m * MACRO:(m + 1) * MACRO, :].rearrange(
                        "(t p) j -> p t j", t=4
                    ),
                    out_sb[:].rearrange("p (t j) -> p t j", t=4),
                )

        for m in range(n_macro):
            ctx1 = emit_pass1(m)
            if pending_pass2[0] is not None:
                emit_pass2(m - 1, *pending_pass2[0])
            pending_pass2[0] = ctx1
        emit_pass2(n_macro - 1, *pending_pass2[0])

    nc.compile()
    return nc


def _get_compiled(bs: int, has_bias: bool):
    key = (bs, has_bias)
    if key not in _compiled:
        _compiled[key] = _build(bs, has_bias)
    return _compiled[key]


def _pack_inputs(agent_obs, messages, Wq, bq, Wk, bk, Wv, bv, Wo, bo):
    """Host-side packing (per full batch): returns dict of device arrays."""
    bf16 = ml_dtypes.bfloat16
    b = agent_obs.shape[0]
    allm = np.concatenate([agent_obs[:, None, :], messages], axis=1)  # [b, 8, 256]
    # slab-chunk-major, feature-transposed: xpack[2s+ch, k, b]
    xpack = np.ascontiguousarray(
        allm.reshape(b, NS, 2, 128).transpose(1, 2, 3, 0).reshape(16, 128, b)
    ).astype(bf16)

    # V (and Wo rows) in d-major column order c' = d*NH + h so the DVE
    # attn broadcast is stride-1 innermost.
    perm = (np.arange(128).reshape(NH, HD).T).reshape(-1)  # c' -> h*HD+d
    WvTp = Wv.T[:, perm]
    wcat = np.concatenate([Wq.T, Wk.T, WvTp], axis=1)  # [256, 384]
    wpack = np.ascontiguousarray(
        wcat.reshape(2, 128, 384).transpose(1, 0, 2)
    ).astype(bf16)  # [128, 2, 384]
    wo = np.ascontiguousarray(Wo.T[perm, :]).astype(bf16)  # [128, 128]

    has_bias = bool(
        np.any(bq != 0) or np.any(bk != 0) or np.any(bv != 0) or np.any(bo != 0)
    )
    extra = {}
    if has_bias:
        # PSUM layout [Q | K0 V0 | ... | K7 V7]
        bkv = np.zeros((1, 2176), np.float32)
        bkv[0, 0:128] = bq
        for s in range(NS):
            bkv[0, 128 + s * 256:128 + s * 256 + 128] = bk
            bkv[0, 256 + s * 256:256 + s * 256 + 128] = bv[perm]
        extra["bkv"] = bkv
        extra["bo"] = bo.reshape(1, 128).astype(np.float32)
    return xpack, wpack, wo, extra, has_bias


def kernel(agent_obs, messages, Wq, bq, Wk, bk, Wv, bv, Wo, bo):
    b = agent_obs.shape[0]
    assert b % N_CORES == 0
    bs = b // N_CORES

    xpack, wpack, wo, extra, has_bias = _pack_inputs(
        np.asarray(agent_obs, np.float32), np.asarray(messages, np.float32),
        np.asarray(Wq, np.float32), np.asarray(bq, np.float32),
        np.asarray(Wk, np.float32), np.asarray(bk, np.float32),
        np.asarray(Wv, np.float32), np.asarray(bv, np.float32),
        np.asarray(Wo, np.float32), np.asarray(bo, np.float32),
    )
    nc = _get_compiled(bs, has_bias)

    in_maps = []
    for c in range(N_CORES):
        m = {
            "xpack": np.ascontiguousarray(xpack[:, :, c * bs:(c + 1) * bs]),
            "wpack": wpack,
            "wo": wo,
        }
        m.update(extra)
        in_maps.append(m)

    res = run_bass_kernel_spmd(nc, in_maps, core_ids=list(range(N_CORES)))
    out = np.concatenate([r["out"] for r in res.results], axis=0)
    return out.astype(np.float32)


# revision 24
# speedup vs baseline: 1.3613x; 1.1662x over previous
"""AttentionCommModule TRN2 kernel: 8-core data-parallel single-query MHA.

Sharding: batch B=32768 split across 8 NeuronCores (4096 rows each); all
weights replicated. Inputs are host-packed to bf16 in slab-chunk-major
layout so each core can xbar-DMA-transpose activation tiles straight from
DRAM into [k, b] SBUF layout (no on-chip transposes).

Per 128-row tile on device (batch-major layout, b on partitions):
  TensorE : Q/K/V projections, lhsT = transposed activation chunk
            (stationary), rhs = packed weights, f32 PSUM accumulation.
  ScalarE : PSUM -> SBUF copies (cast to bf16), exp().
  VectorE : QK dot-products + halving-tree d-reduction, softmax
            (batched per 4 tiles), attn*V + n-reduction.
  out-proj: weighted tile xbar-transposed (SBUF->SBUF DMA), one matmul
            against Wo^T, result staged in SBUF, DMA'd out per macro-tile.
"""

import numpy as np
import ml_dtypes
from contextlib import ExitStack

import concourse.bass as bass
import concourse.tile as tile
from concourse import bacc, mybir
from concourse.bass_utils import run_bass_kernel_spmd

N_CORES = 8
B_FULL = 32768
INPUT_DIM = 256
COMM = 128
NH = 4
HD = 32
N_MSGS = 7
NS = 8          # slabs = num_agents (self + 7 messages)
TILE = 128      # rows per attention tile
MACRO = 512     # rows per DMA macro-tile

BF = mybir.dt.bfloat16
F32 = mybir.dt.float32
INV_SQRT_HD = 1.0 / float(np.sqrt(HD))
PSUM_OUT_DMA = False  # PSUM is not DMA-addressable on this stack

_compiled = {}


def _build(bs: int, has_bias: bool):
    """Build + compile the per-core Bass program for a bs-row shard."""
    assert bs % MACRO == 0
    nc = bacc.Bacc(
        "TRN2",
        target_bir_lowering=False,
        debug=False,
        enable_asserts=False,
        num_devices=N_CORES,
    )
    # ktpack[c, k, b]: slab-chunk c, feature k on what becomes the SBUF
    # partition dim, batch contiguous — host pre-transposed.
    xpack = nc.dram_tensor("xpack", [2 * NS, 128, bs], BF, kind="ExternalInput").ap()
    wpack = nc.dram_tensor("wpack", [128, 2, 384], BF, kind="ExternalInput").ap()
    wod = nc.dram_tensor("wo", [128, 128], BF, kind="ExternalInput").ap()
    identd = nc.dram_tensor("ident", [128, 128], BF, kind="ExternalInput").ap()
    if has_bias:
        bkvd = nc.dram_tensor("bkv", [1, 2176], F32, kind="ExternalInput").ap()
        bod = nc.dram_tensor("bo", [1, 128], F32, kind="ExternalInput").ap()
    outd = nc.dram_tensor("out", [bs, 128], F32, kind="ExternalOutput").ap()

    with tile.TileContext(nc) as tc, ExitStack() as ctx:
        consts = ctx.enter_context(tc.tile_pool(name="consts", bufs=1))
        xtp = ctx.enter_context(tc.tile_pool(name="xtp", bufs=3))
        kvp = ctx.enter_context(tc.tile_pool(name="kvp", bufs=14))
        work = ctx.enter_context(tc.tile_pool(name="work", bufs=4))
        sm = ctx.enter_context(tc.tile_pool(name="sm", bufs=4))
        osb = ctx.enter_context(tc.tile_pool(name="osb", bufs=2))
        psum = ctx.enter_context(tc.tile_pool(name="psum", bufs=8, space="PSUM"))

        w_sb = consts.tile([128, 768], BF)
        nc.gpsimd.dma_start(w_sb[:].rearrange("p (c n) -> p c n", c=2), wpack[:, :, :])
        wo_sb = consts.tile([128, 128], BF)
        nc.gpsimd.dma_start(wo_sb[:], wod[:, :])
        ident_sb = consts.tile([128, 128], BF)
        nc.gpsimd.dma_start(ident_sb[:], identd[:, :])
        if has_bias:
            bkv_sb = consts.tile([1, 2176], F32)
            nc.gpsimd.dma_start(bkv_sb[:], bkvd[:, :])
            bo_sb = consts.tile([1, 128], F32)
            nc.gpsimd.dma_start(bo_sb[:], bod[:, :])

        n_macro = bs // MACRO
        # Software pipeline, 2 macros deep, interleaved at tile granularity:
        # each engine's in-order stream alternates [pass2-tile(m-2, j),
        # pass1-tile(m, j)] so no pass-2 dependency (DVE tail -> xbar ->
        # out-proj matmul) ever blocks the next macro's projection work.
        def emit_load(m):
            # ---- input load: 16 plain DMAs, already [k, b] in DRAM ----
            xt = xtp.tile([128, 16 * MACRO], BF, name=f"xt{m}", tag="xt")
            for c in range(16):
                nc.scalar.dma_start(
                    xt[:, c * MACRO:(c + 1) * MACRO],
                    xpack[c, :, m * MACRO:(m + 1) * MACRO],
                )
            return xt

        def emit_pass1_tile(m, j, xt, scores4, kv2s):
            # PSUM layout: [Q | K0 V0 | K1 V1 ... K7 V7]
            psA = psum.tile([128, 384], F32, tag="ps", name=f"psA{m}_{j}")
            for ch in (0, 1):
                nc.tensor.matmul(
                    psA[:, :],
                    lhsT=xt[:, ch * MACRO + j * TILE: ch * MACRO + j * TILE + 128],
                    rhs=w_sb[:, ch * 384:(ch + 1) * 384],
                    start=(ch == 0),
                    stop=(ch == 1),
                )
            kvtiles = [psA]
            for pi, pair in enumerate(((1, 2), (3, 4), (5, 6), (7,))):
                width = 256 * len(pair)
                ps = psum.tile([128, width], F32, tag="ps", name=f"ps{pi}_{m}_{j}")
                for si, s in enumerate(pair):
                    for ch in (0, 1):
                        nc.tensor.matmul(
                            ps[:, si * 256:(si + 1) * 256],
                            lhsT=xt[:, (2 * s + ch) * MACRO + j * TILE:
                                    (2 * s + ch) * MACRO + j * TILE + 128],
                            rhs=w_sb[:, ch * 384 + 128:(ch + 1) * 384],
                            start=(ch == 0),
                            stop=(ch == 1),
                        )
                kvtiles.append(ps)

            kv2 = kvp.tile([128, 2176], BF, tag="kv2", name=f"kv2_{m}_{j}")
            kv2s.append(kv2)
            off = 0
            for ps in kvtiles:
                w = ps.shape[1]
                nc.scalar.copy(kv2[:, off:off + w], ps[:, :])
                off += w
            if has_bias:
                nc.vector.tensor_add(
                    kv2[:, :], kv2[:, :], bkv_sb[:, :].partition_broadcast(128)
                )

            # K-products: q (bcast over slabs) * K_s
            qb = (
                kv2[:, 0:128]
                .rearrange("p (h d) -> p h d", h=NH)
                .unsqueeze(1)
                .broadcast_to([128, NS, NH, HD])
            )
            kk = kv2[:, 128:2176].rearrange(
                "p (s kv h d) -> p s kv h d", s=NS, kv=2, h=NH
            )[:, :, 0, :, :]
            p1 = work.tile([128, 1024], BF, tag="p1", name=f"p1_{m}_{j}")
            nc.vector.tensor_mul(
                p1[:].rearrange("p (s h d) -> p s h d", s=NS, h=NH), qb, kk
            )
            # d-reduction tree: 32 -> 1 per (s, h)
            cur, cd = p1, HD
            for r in range(4):
                nxt = work.tile(
                    [128, NS * NH * cd // 2], BF, tag=f"t{r}", name=f"t{r}_{m}_{j}"
                )
                v = cur[:].rearrange(
                    "p (s h e d) -> p s h e d", s=NS, h=NH, e=2
                )
                nc.vector.tensor_add(
                    nxt[:].rearrange("p (s h d) -> p s h d", s=NS, h=NH),
                    v[:, :, :, 0, :],
                    v[:, :, :, 1, :],
                )
                cur, cd = nxt, cd // 2
            v = cur[:].rearrange("p (s h e) -> p s h e", s=NS, h=NH)
            nc.vector.tensor_add(
                scores4[:, j * 32:(j + 1) * 32].rearrange(
                    "p (s h) -> p s h", s=NS
                ),
                v[:, :, :, 0],
                v[:, :, :, 1],
            )

        def emit_softmax(m, scores4):
            # ---- softmax over slabs, batched for the 4 tiles ----
            # scores4 layout: (t, s, h) per partition
            sc_tsh = scores4[:].rearrange("p (t s h) -> p t s h", t=4, s=NS)
            sc_ths = sc_tsh.transpose([0, 1, 3, 2])
            mx = sm.tile([128, 16], F32, tag="mx")
            nc.vector.reduce_max(mx[:].rearrange("p (t h) -> p t h", t=4),
                                 sc_ths, axis=mybir.AxisListType.X)
            u4 = sm.tile([128, 128], F32, tag="u4")
            mxb = (
                mx[:]
                .rearrange("p (t h) -> p t h", t=4)
                .unsqueeze(2)
                .broadcast_to([128, 4, NS, NH])
            )
            nc.vector.tensor_sub(
                u4[:].rearrange("p (t s h) -> p t s h", t=4, s=NS), sc_tsh, mxb
            )
            e4 = sm.tile([128, 128], BF, tag="e4")
            nc.scalar.activation(
                e4[:], u4[:], mybir.ActivationFunctionType.Exp, scale=INV_SQRT_HD
            )
            s4 = sm.tile([128, 16], F32, tag="s4")
            e_tsh = e4[:].rearrange("p (t s h) -> p t s h", t=4, s=NS)
            nc.vector.reduce_sum(s4[:].rearrange("p (t h) -> p t h", t=4),
                                 e_tsh.transpose([0, 1, 3, 2]),
                                 axis=mybir.AxisListType.X)
            r4 = sm.tile([128, 16], F32, tag="r4")
            nc.vector.reciprocal(r4[:], s4[:])
            a4 = sm.tile([128, 128], BF, tag="a4")
            r4b = (
                r4[:]
                .rearrange("p (t h) -> p t h", t=4)
                .unsqueeze(2)
                .broadcast_to([128, 4, NS, NH])
            )
            nc.vector.tensor_mul(
                a4[:].rearrange("p (t s h) -> p t s h", t=4, s=NS), e_tsh, r4b
            )
            return a4

        def emit_pass2a_tile(m, j, kv2, a4):
            # ---- pass 2a: attn * V, n-reduction, transpose of weighted ----
            # V columns are d-major (c' = d*4 + h, host-permuted) so the
            # attn broadcast lands on a stride-1 innermost dim (DVE 2x).
            ab = (
                a4[:, j * 32:(j + 1) * 32]
                .rearrange("p (s h) -> p s h", s=NS)
                .unsqueeze(2)
                .broadcast_to([128, NS, HD, NH])
            )
            vv = kv2[:, 128:2176].rearrange(
                "p (s kv d h) -> p s kv d h", s=NS, kv=2, d=HD
            )[:, :, 1, :, :]
            p2 = work.tile([128, 1024], BF, tag="p2", name=f"p2_{m}_{j}")
            nc.vector.tensor_mul(
                p2[:].rearrange("p (s d h) -> p s d h", s=NS, d=HD), ab, vv
            )
            cur, cn = p2, NS
            for r in range(3):
                nxt = work.tile(
                    [128, cn * 64], BF, tag=f"v{r}", name=f"v{r}_{m}_{j}"
                )
                nc.vector.tensor_add(
                    nxt[:], cur[:, 0:cn * 64], cur[:, cn * 64:cn * 128]
                )
                cur, cn = nxt, cn // 2
            wtd = cur  # [128, 128] bf16, batch-major, (d, h) cols
            ptp = psum.tile([128, 128], BF, tag="ps", name=f"ptp_{m}_{j}")
            nc.tensor.transpose(ptp[:], wtd[:], ident_sb[:])
            wtdT = work.tile([128, 128], BF, tag="wtdT", name=f"wtdT_{m}_{j}")
            nc.scalar.copy(wtdT[:], ptp[:])
            return wtdT

        def emit_pass2b_tile(m, j, wtdT, out_sb):
            po = psum.tile([128, 128], F32, tag="ps", name=f"po_{m}_{j}")
            nc.tensor.matmul(po[:], lhsT=wtdT[:], rhs=wo_sb[:],
                             start=True, stop=True)
            r0 = (m * 4 + j) * TILE
            if has_bias:
                ob = osb.tile([128, 128], F32, tag="ob", name=f"ob_{m}_{j}")
                nc.vector.tensor_add(
                    ob[:], po[:], bo_sb[:, :].partition_broadcast(128)
                )
                nc.gpsimd.dma_start(outd[r0:r0 + TILE, :], ob[:])
            else:
                nc.scalar.copy(out_sb[:, j * 128:(j + 1) * 128], po[:])

        def emit_out_dma(m, out_sb):
            if not has_bias:
                nc.gpsimd.dma_start(
                    outd[m * MACRO:(m + 1) * MACRO, :].rearrange(
                        "(t p) j -> p t j", t=4
                    ),
                    out_sb[:].rearrange("p (t j) -> p t j", t=4),
                )

        DEPTH = 2
        state = {}
        for m in range(n_macro + DEPTH):
            if m < n_macro:
                st = state[m] = {
                    "xt": emit_load(m),
                    "scores4": sm.tile([128, 128], F32, tag="scores4",
                                       name=f"sc4_{m}"),
                    "kv2s": [],
                    "out_sb": None,
                }
            for j in range(4):
                if m - DEPTH >= 0:
                    old = state[m - DEPTH]
                    if j == 0 and not has_bias:
                        old["out_sb"] = osb.tile(
                            [128, 4 * TILE], F32, tag="out_sb",
                            name=f"osb{m - DEPTH}")
                    wtdT = emit_pass2a_tile(m - DEPTH, j, old["kv2s"][j],
                                            old["a4"])
                if m < n_macro:
                    emit_pass1_tile(m, j, st["xt"], st["scores4"], st["kv2s"])
                if m - DEPTH >= 0:
                    emit_pass2b_tile(m - DEPTH, j, wtdT, old["out_sb"])
            if m - DEPTH >= 0:
                emit_out_dma(m - DEPTH, state[m - DEPTH]["out_sb"])
                del state[m - DEPTH]
            if m < n_macro:
                st["a4"] = emit_softmax(m, st["scores4"])

    nc.compile()
    return nc


def _get_compiled(bs: int, has_bias: bool):
    key = (bs, has_bias)
    if key not in _compiled:
        _compiled[key] = _build(bs, has_bias)
    return _compiled[key]


def _pack_inputs(agent_obs, messages, Wq, bq, Wk, bk, Wv, bv, Wo, bo):
    """Host-side packing (per full batch): returns dict of device arrays."""
    bf16 = ml_dtypes.bfloat16
    b = agent_obs.shape[0]
    allm = np.concatenate([agent_obs[:, None, :], messages], axis=1)  # [b, 8, 256]
    # slab-chunk-major, feature-transposed: xpack[2s+ch, k, b]
    xpack = np.ascontiguousarray(
        allm.reshape(b, NS, 2, 128).transpose(1, 2, 3, 0).reshape(16, 128, b)
    ).astype(bf16)

    # V (and Wo rows) in d-major column order c' = d*NH + h so the DVE
    # attn broadcast is stride-1 innermost.
    perm = (np.arange(128).reshape(NH, HD).T).reshape(-1)  # c' -> h*HD+d
    WvTp = Wv.T[:, perm]
    wcat = np.concatenate([Wq.T, Wk.T, WvTp], axis=1)  # [256, 384]
    wpack = np.ascontiguousarray(
        wcat.reshape(2, 128, 384).transpose(1, 0, 2)
    ).astype(bf16)  # [128, 2, 384]
    wo = np.ascontiguousarray(Wo.T[perm, :]).astype(bf16)  # [128, 128]

    has_bias = bool(
        np.any(bq != 0) or np.any(bk != 0) or np.any(bv != 0) or np.any(bo != 0)
    )
    extra = {"ident": np.eye(128, dtype=bf16)}
    if has_bias:
        # PSUM layout [Q | K0 V0 | ... | K7 V7]
        bkv = np.zeros((1, 2176), np.float32)
        bkv[0, 0:128] = bq
        for s in range(NS):
            bkv[0, 128 + s * 256:128 + s * 256 + 128] = bk
            bkv[0, 256 + s * 256:256 + s * 256 + 128] = bv[perm]
        extra["bkv"] = bkv
        extra["bo"] = bo.reshape(1, 128).astype(np.float32)
    return xpack, wpack, wo, extra, has_bias


def kernel(agent_obs, messages, Wq, bq, Wk, bk, Wv, bv, Wo, bo):
    b = agent_obs.shape[0]
    assert b % N_CORES == 0
    bs = b // N_CORES

    xpack, wpack, wo, extra, has_bias = _pack_inputs(
        np.asarray(agent_obs, np.float32), np.asarray(messages, np.float32),
        np.asarray(Wq, np.float32), np.asarray(bq, np.float32),
        np.asarray(Wk, np.float32), np.asarray(bk, np.float32),
        np.asarray(Wv, np.float32), np.asarray(bv, np.float32),
        np.asarray(Wo, np.float32), np.asarray(bo, np.float32),
    )
    nc = _get_compiled(bs, has_bias)

    in_maps = []
    for c in range(N_CORES):
        m = {
            "xpack": np.ascontiguousarray(xpack[:, :, c * bs:(c + 1) * bs]),
            "wpack": wpack,
            "wo": wo,
        }
        m.update(extra)
        in_maps.append(m)

    res = run_bass_kernel_spmd(nc, in_maps, core_ids=list(range(N_CORES)))
    out = np.concatenate([r["out"] for r in res.results], axis=0)
    return out.astype(np.float32)


# revision 31
# speedup vs baseline: 1.5608x; 1.1466x over previous
"""AttentionCommModule TRN2 kernel: 8-core data-parallel single-query MHA.

Sharding: batch B=32768 split across 8 NeuronCores (4096 rows each); all
weights replicated. Inputs are host-packed to bf16 in slab-chunk-major
layout so each core can xbar-DMA-transpose activation tiles straight from
DRAM into [k, b] SBUF layout (no on-chip transposes).

Per 128-row tile on device (batch-major layout, b on partitions):
  TensorE : Q/K/V projections, lhsT = transposed activation chunk
            (stationary), rhs = packed weights, f32 PSUM accumulation.
  ScalarE : PSUM -> SBUF copies (cast to bf16), exp().
  VectorE : QK dot-products + halving-tree d-reduction, softmax
            (batched per 4 tiles), attn*V + n-reduction.
  out-proj: weighted tile xbar-transposed (SBUF->SBUF DMA), one matmul
            against Wo^T, result staged in SBUF, DMA'd out per macro-tile.
"""

import numpy as np
import ml_dtypes
from contextlib import ExitStack

import concourse.bass as bass
import concourse.tile as tile
from concourse import bacc, mybir
from concourse.bass_utils import run_bass_kernel_spmd

N_CORES = 8
B_FULL = 32768
INPUT_DIM = 256
COMM = 128
NH = 4
HD = 32
N_MSGS = 7
NS = 8          # slabs = num_agents (self + 7 messages)
TILE = 128      # rows per attention tile
MACRO = 512     # rows per DMA macro-tile

BF = mybir.dt.bfloat16
F32 = mybir.dt.float32
INV_SQRT_HD = 1.0 / float(np.sqrt(HD))
PSUM_OUT_DMA = False  # PSUM is not DMA-addressable on this stack

_compiled = {}


def _build(bs: int, has_bias: bool):
    """Build + compile the per-core Bass program for a bs-row shard."""
    assert bs % MACRO == 0
    nc = bacc.Bacc(
        "TRN2",
        target_bir_lowering=False,
        debug=False,
        enable_asserts=False,
        num_devices=N_CORES,
    )
    # ktpack[c, k, b]: slab-chunk c, feature k on what becomes the SBUF
    # partition dim, batch contiguous — host pre-transposed.
    xpack = nc.dram_tensor("xpack", [2 * NS, 128, bs], BF, kind="ExternalInput").ap()
    wpack = nc.dram_tensor("wpack", [128, 2, 384], BF, kind="ExternalInput").ap()
    wod = nc.dram_tensor("wo", [128, 128], BF, kind="ExternalInput").ap()
    identd = nc.dram_tensor("ident", [128, 128], BF, kind="ExternalInput").ap()
    if has_bias:
        bkvd = nc.dram_tensor("bkv", [1, 2176], F32, kind="ExternalInput").ap()
        bod = nc.dram_tensor("bo", [1, 128], F32, kind="ExternalInput").ap()
    outd = nc.dram_tensor("out", [bs, 128], F32, kind="ExternalOutput").ap()

    with tile.TileContext(nc) as tc, ExitStack() as ctx:
        consts = ctx.enter_context(tc.tile_pool(name="consts", bufs=1))
        xtp = ctx.enter_context(tc.tile_pool(name="xtp", bufs=3))
        kvp = ctx.enter_context(tc.tile_pool(name="kvp", bufs=14))
        work = ctx.enter_context(tc.tile_pool(name="work", bufs=4))
        sm = ctx.enter_context(tc.tile_pool(name="sm", bufs=4))
        osb = ctx.enter_context(tc.tile_pool(name="osb", bufs=2))
        psum = ctx.enter_context(tc.tile_pool(name="psum", bufs=8, space="PSUM"))

        w_sb = consts.tile([128, 768], BF)
        nc.gpsimd.dma_start(w_sb[:].rearrange("p (c n) -> p c n", c=2), wpack[:, :, :])
        wo_sb = consts.tile([128, 128], BF)
        nc.gpsimd.dma_start(wo_sb[:], wod[:, :])
        ident_sb = consts.tile([128, 128], BF)
        nc.gpsimd.dma_start(ident_sb[:], identd[:, :])
        if has_bias:
            bkv_sb = consts.tile([1, 2176], F32)
            nc.gpsimd.dma_start(bkv_sb[:], bkvd[:, :])
            bo_sb = consts.tile([1, 128], F32)
            nc.gpsimd.dma_start(bo_sb[:], bod[:, :])

        n_macro = bs // MACRO
        # Software pipeline, 2 macros deep, interleaved at tile granularity:
        # each engine's in-order stream alternates [pass2-tile(m-2, j),
        # pass1-tile(m, j)] so no pass-2 dependency (DVE tail -> xbar ->
        # out-proj matmul) ever blocks the next macro's projection work.
        def emit_load(m):
            # ---- input load: 16 plain DMAs, already [k, b] in DRAM ----
            xt = xtp.tile([128, 16 * MACRO], BF, name=f"xt{m}", tag="xt")
            for c in range(16):
                nc.scalar.dma_start(
                    xt[:, c * MACRO:(c + 1) * MACRO],
                    xpack[c, :, m * MACRO:(m + 1) * MACRO],
                )
            return xt

        def emit_pass1_tile(m, j, xt, scores4, kv2pairs):
            # PSUM layout: [Q | K0 V0 | K1 V1 ... K7 V7]
            psA = psum.tile([128, 384], F32, tag="ps", name=f"psA{m}_{j}")
            for ch in (0, 1):
                nc.tensor.matmul(
                    psA[:, :],
                    lhsT=xt[:, ch * MACRO + j * TILE: ch * MACRO + j * TILE + 128],
                    rhs=w_sb[:, ch * 384:(ch + 1) * 384],
                    start=(ch == 0),
                    stop=(ch == 1),
                )
            kvtiles = [psA]
            for pi, pair in enumerate(((1, 2), (3, 4), (5, 6), (7,))):
                width = 256 * len(pair)
                ps = psum.tile([128, width], F32, tag="ps", name=f"ps{pi}_{m}_{j}")
                for si, s in enumerate(pair):
                    for ch in (0, 1):
                        nc.tensor.matmul(
                            ps[:, si * 256:(si + 1) * 256],
                            lhsT=xt[:, (2 * s + ch) * MACRO + j * TILE:
                                    (2 * s + ch) * MACRO + j * TILE + 128],
                            rhs=w_sb[:, ch * 384 + 128:(ch + 1) * 384],
                            start=(ch == 0),
                            stop=(ch == 1),
                        )
                kvtiles.append(ps)

            kv2 = kvp.tile([128, 2176], BF, tag="kv2", name=f"kv2_{m}_{j}")
            kv2pairs.append(kv2)
            off = 0
            for ps in kvtiles:
                w = ps.shape[1]
                nc.scalar.copy(kv2[:, off:off + w], ps[:, :])
                off += w
            if has_bias:
                nc.vector.tensor_add(
                    kv2[:, :], kv2[:, :], bkv_sb[:, :].partition_broadcast(128)
                )

            # K-products: q (bcast over slabs) * K_s
            qb = (
                kv2[:, 0:128]
                .rearrange("p (h d) -> p h d", h=NH)
                .unsqueeze(1)
                .broadcast_to([128, NS, NH, HD])
            )
            kk = kv2[:, 128:2176].rearrange(
                "p (s kv h d) -> p s kv h d", s=NS, kv=2, h=NH
            )[:, :, 0, :, :]
            p1 = work.tile([128, 1024], BF, tag="p1", name=f"p1_{m}_{j}")
            nc.vector.tensor_mul(
                p1[:].rearrange("p (s h d) -> p s h d", s=NS, h=NH), qb, kk
            )
            # d-reduction tree: 32 -> 1 per (s, h)
            cur, cd = p1, HD
            for r in range(4):
                nxt = work.tile(
                    [128, NS * NH * cd // 2], BF, tag=f"t{r}", name=f"t{r}_{m}_{j}"
                )
                v = cur[:].rearrange(
                    "p (s h e d) -> p s h e d", s=NS, h=NH, e=2
                )
                nc.vector.tensor_add(
                    nxt[:].rearrange("p (s h d) -> p s h d", s=NS, h=NH),
                    v[:, :, :, 0, :],
                    v[:, :, :, 1, :],
                )
                cur, cd = nxt, cd // 2
            v = cur[:].rearrange("p (s h e) -> p s h e", s=NS, h=NH)
            nc.vector.tensor_add(
                scores4[:, j * 32:(j + 1) * 32].rearrange(
                    "p (s h) -> p s h", s=NS
                ),
                v[:, :, :, 0],
                v[:, :, :, 1],
            )

        def emit_softmax(m, scores4):
            # ---- softmax over slabs, batched for the 4 tiles ----
            # scores4 layout: (t, s, h) per partition
            sc_tsh = scores4[:].rearrange("p (t s h) -> p t s h", t=4, s=NS)
            sc_ths = sc_tsh.transpose([0, 1, 3, 2])
            mx = sm.tile([128, 16], F32, tag="mx")
            nc.vector.reduce_max(mx[:].rearrange("p (t h) -> p t h", t=4),
                                 sc_ths, axis=mybir.AxisListType.X)
            u4 = sm.tile([128, 128], F32, tag="u4")
            mxb = (
                mx[:]
                .rearrange("p (t h) -> p t h", t=4)
                .unsqueeze(2)
                .broadcast_to([128, 4, NS, NH])
            )
            nc.vector.tensor_sub(
                u4[:].rearrange("p (t s h) -> p t s h", t=4, s=NS), sc_tsh, mxb
            )
            e4 = sm.tile([128, 128], BF, tag="e4")
            nc.scalar.activation(
                e4[:], u4[:], mybir.ActivationFunctionType.Exp, scale=INV_SQRT_HD
            )
            s4 = sm.tile([128, 16], F32, tag="s4")
            e_tsh = e4[:].rearrange("p (t s h) -> p t s h", t=4, s=NS)
            nc.vector.reduce_sum(s4[:].rearrange("p (t h) -> p t h", t=4),
                                 e_tsh.transpose([0, 1, 3, 2]),
                                 axis=mybir.AxisListType.X)
            r4 = sm.tile([128, 16], F32, tag="r4")
            nc.vector.reciprocal(r4[:], s4[:])
            a4 = sm.tile([128, 128], BF, tag="a4")
            r4b = (
                r4[:]
                .rearrange("p (t h) -> p t h", t=4)
                .unsqueeze(2)
                .broadcast_to([128, 4, NS, NH])
            )
            nc.vector.tensor_mul(
                a4[:].rearrange("p (t s h) -> p t s h", t=4, s=NS), e_tsh, r4b
            )
            return a4

        def emit_pass2a_tile(m, j, kv2, a4):
            # ---- pass 2a: attn * V, n-reduction, transpose of weighted ----
            # V columns are d-major (c' = d*4 + h, host-permuted) so the
            # attn broadcast lands on a stride-1 innermost dim (DVE 2x).
            ab = (
                a4[:, j * 32:(j + 1) * 32]
                .rearrange("p (s h) -> p s h", s=NS)
                .unsqueeze(2)
                .broadcast_to([128, NS, HD, NH])
            )
            vv = kv2[:, 128:2176].rearrange(
                "p (s kv d h) -> p s kv d h", s=NS, kv=2, d=HD
            )[:, :, 1, :, :]
            p2 = work.tile([128, 1024], BF, tag="p2", name=f"p2_{m}_{j}")
            nc.vector.tensor_mul(
                p2[:].rearrange("p (s d h) -> p s d h", s=NS, d=HD), ab, vv
            )
            cur, cn = p2, NS
            for r in range(3):
                nxt = work.tile(
                    [128, cn * 64], BF, tag=f"v{r}", name=f"v{r}_{m}_{j}"
                )
                nc.vector.tensor_add(
                    nxt[:], cur[:, 0:cn * 64], cur[:, cn * 64:cn * 128]
                )
                cur, cn = nxt, cn // 2
            wtd = cur  # [128, 128] bf16, batch-major, (d, h) cols
            ptp = psum.tile([128, 128], BF, tag="ps", name=f"ptp_{m}_{j}")
            nc.tensor.transpose(ptp[:], wtd[:], ident_sb[:])
            wtdT = work.tile([128, 128], BF, tag="wtdT", name=f"wtdT_{m}_{j}")
            nc.scalar.copy(wtdT[:], ptp[:])
            return wtdT

        def emit_pass2b_tile(m, j, wtdT, out_sb):
            po = psum.tile([128, 128], F32, tag="ps", name=f"po_{m}_{j}")
            nc.tensor.matmul(po[:], lhsT=wtdT[:], rhs=wo_sb[:],
                             start=True, stop=True)
            r0 = (m * 4 + j) * TILE
            if has_bias:
                ob = osb.tile([128, 128], F32, tag="ob", name=f"ob_{m}_{j}")
                nc.vector.tensor_add(
                    ob[:], po[:], bo_sb[:, :].partition_broadcast(128)
                )
                nc.gpsimd.dma_start(outd[r0:r0 + TILE, :], ob[:])
            else:
                nc.scalar.copy(out_sb[:, j * 128:(j + 1) * 128], po[:])

        def emit_out_dma(m, out_sb):
            if not has_bias:
                nc.gpsimd.dma_start(
                    outd[m * MACRO:(m + 1) * MACRO, :].rearrange(
                        "(t p) j -> p t j", t=4
                    ),
                    out_sb[:].rearrange("p (t j) -> p t j", t=4),
                )

        DEPTH = 2
        state = {}
        for m in range(n_macro + DEPTH):
            if m < n_macro:
                st = state[m] = {
                    "xt": emit_load(m),
                    "scores4": sm.tile([128, 128], F32, tag="scores4",
                                       name=f"sc4_{m}"),
                    "kv2pairs": [],
                    "out_sb": None,
                }
            for j in range(4):
                if m - DEPTH >= 0:
                    old = state[m - DEPTH]
                    if j == 0 and not has_bias:
                        old["out_sb"] = osb.tile(
                            [128, 4 * TILE], F32, tag="out_sb",
                            name=f"osb{m - DEPTH}")
                    wtdT = emit_pass2a_tile(m - DEPTH, j, old["kv2pairs"][j],
                                            old["a4"])
                if m < n_macro:
                    emit_pass1_tile(m, j, st["xt"], st["scores4"],
                                    st["kv2pairs"])
                if m - DEPTH >= 0:
                    emit_pass2b_tile(m - DEPTH, j, wtdT, old["out_sb"])
            if m - DEPTH >= 0:
                emit_out_dma(m - DEPTH, state[m - DEPTH]["out_sb"])
                del state[m - DEPTH]
            if m < n_macro:
                st["a4"] = emit_softmax(m, st["scores4"])

    nc.compile()
    return nc


def _get_compiled(bs: int, has_bias: bool):
    key = (bs, has_bias)
    if key not in _compiled:
        _compiled[key] = _build(bs, has_bias)
    return _compiled[key]


def _pack_inputs(agent_obs, messages, Wq, bq, Wk, bk, Wv, bv, Wo, bo):
    """Host-side packing (per full batch): returns dict of device arrays."""
    bf16 = ml_dtypes.bfloat16
    b = agent_obs.shape[0]
    allm = np.concatenate([agent_obs[:, None, :], messages], axis=1)  # [b, 8, 256]
    # slab-chunk-major, feature-transposed: xpack[2s+ch, k, b]
    xpack = np.ascontiguousarray(
        allm.reshape(b, NS, 2, 128).transpose(1, 2, 3, 0).reshape(16, 128, b)
    ).astype(bf16)

    # V (and Wo rows) in d-major column order c' = d*NH + h so the DVE
    # attn broadcast is stride-1 innermost.
    perm = (np.arange(128).reshape(NH, HD).T).reshape(-1)  # c' -> h*HD+d
    WvTp = Wv.T[:, perm]
    wcat = np.concatenate([Wq.T, Wk.T, WvTp], axis=1)  # [256, 384]
    wpack = np.ascontiguousarray(
        wcat.reshape(2, 128, 384).transpose(1, 0, 2)
    ).astype(bf16)  # [128, 2, 384]
    wo = np.ascontiguousarray(Wo.T[perm, :]).astype(bf16)  # [128, 128]

    has_bias = bool(
        np.any(bq != 0) or np.any(bk != 0) or np.any(bv != 0) or np.any(bo != 0)
    )
    extra = {"ident": np.eye(128, dtype=bf16)}
    if has_bias:
        # PSUM layout [Q | K0 V0 | ... | K7 V7]
        bkv = np.zeros((1, 2176), np.float32)
        bkv[0, 0:128] = bq
        for s in range(NS):
            bkv[0, 128 + s * 256:128 + s * 256 + 128] = bk
            bkv[0, 256 + s * 256:256 + s * 256 + 128] = bv[perm]
        extra["bkv"] = bkv
        extra["bo"] = bo.reshape(1, 128).astype(np.float32)
    return xpack, wpack, wo, extra, has_bias


def kernel(agent_obs, messages, Wq, bq, Wk, bk, Wv, bv, Wo, bo):
    b = agent_obs.shape[0]
    assert b % N_CORES == 0
    bs = b // N_CORES

    xpack, wpack, wo, extra, has_bias = _pack_inputs(
        np.asarray(agent_obs, np.float32), np.asarray(messages, np.float32),
        np.asarray(Wq, np.float32), np.asarray(bq, np.float32),
        np.asarray(Wk, np.float32), np.asarray(bk, np.float32),
        np.asarray(Wv, np.float32), np.asarray(bv, np.float32),
        np.asarray(Wo, np.float32), np.asarray(bo, np.float32),
    )
    nc = _get_compiled(bs, has_bias)

    in_maps = []
    for c in range(N_CORES):
        m = {
            "xpack": np.ascontiguousarray(xpack[:, :, c * bs:(c + 1) * bs]),
            "wpack": wpack,
            "wo": wo,
        }
        m.update(extra)
        in_maps.append(m)

    res = run_bass_kernel_spmd(nc, in_maps, core_ids=list(range(N_CORES)))
    out = np.concatenate([r["out"] for r in res.results], axis=0)
    return out.astype(np.float32)
